# revision 26
# baseline (speedup 1.0000x reference)
"""Trainium2 Bass kernel for deformable 3x3 convolution (nn_DeformConvWarp).

Problem: x [4,128,128,128] f32, offset [4,18,128,128] f32 (torchvision layout,
per-tap (dy,dx) interleaved), weight [128,128,3,3] f32.
out[b,o,h,w] = sum_{c,k} W[o,c,k] * bilinear_sample(x[b,c], p_k(h,w)+off_k(h,w))

Sharding: 8 cores = batch (4) x output-row-half (2). Each core computes
out[b, :, h2*64:(h2+1)*64, :] = [128, 8192] f32.

Design (v3):
  - Host precomputes, per core: gather indices (one per (pixel, tap)) into a
    row-pair-interleaved image copy xt2 (row i = [xq[i], xq[i+128]], xq = the
    zero-padded NHWC image), so a single 1KB contiguous chunk fetches all 4
    bilinear neighbors (TL, BL, TR, BR); the 4 bilinear weights (validity
    folded in, so out-of-range taps contribute 0); and the SWDGE wrapped
    int16 index tensor.
  - Device: per 512-pixel tile, ONE dma_gather call (4608 idxs, 289
    descriptors) -> g [128 pix, 36 (k,b), 4x128ch]; DVE scales the 4 neighbor
    sections in place (bf16 2x mode via 8-expanded weights); PE transposes
    the scaled sections (matmul vs identity) accumulating the bilinear sum in
    PSUM -> patches [c, k, pix]; PE conv: 9 matmuls accumulate W_k^T @
    patches_k; out DMA per tile.
  - Few large gather calls (16/core) avoid the ~0.6-1us/call SWDGE fixed
    overhead that dominated the previous version (576 calls, 395us GpSimd
    busy); gather DMA traffic (75.5MB/core) is the expected bottleneck.
"""

import os
import sys
import numpy as np

sys.path.insert(0, "/opt/trn_rl_repo")

import ml_dtypes

bf16 = ml_dtypes.bfloat16

B, C, H, W = 4, 128, 128, 128
O, K = 128, 9
HALF = 64
NPIX = HALF * W          # 8192 pixels per core
NBLK = HALF              # 64 row-blocks of 128 pixels
TBLK = 4                 # row-blocks per tile
NT = NBLK // TBLK        # 16 tiles
TPIX = TBLK * 128        # 512 pixels per tile
HW = H * W
KB = K * TBLK            # 36 (k, b) pairs per tile
NIDX_T = KB * 128        # 4608 gather idxs per tile
XQPAD = 129              # leading zero slots in xq
XT2ROWS = 16516          # covers idx <= 16513 (+1 row for the 2-row chunk)

_CACHE = {}


def _build_nc():
    import concourse.bass as bass
    import concourse.mybir as mybir
    import concourse.tile as tile
    from concourse import bacc

    f32 = mybir.dt.float32
    bft = mybir.dt.bfloat16
    i16 = mybir.dt.int16
    Alu = mybir.AluOpType

    nc = bacc.Bacc("TRN2", target_bir_lowering=False, debug=False,
                   num_swdge_queues=4)

    xt2 = nc.declare_dram_parameter("xt2", [XT2ROWS, 2 * C], bft, isOutput=False)
    wrap = nc.declare_dram_parameter("wrap", [128, NT * (NIDX_T // 16)], i16,
                                     isOutput=False)
    # compact bilinear weights, one per neighbor section (TL, BL, TR, BR):
    # ac[i][p, t, k*TBLK+b] = weight for pixel p of block t*TBLK+b, tap k
    ac = nc.declare_dram_parameter("ac", [4, 128, NT, KB], bft, isOutput=False)
    wt = nc.declare_dram_parameter("wt", [K, C, O], bft, isOutput=False)
    identb = nc.declare_dram_parameter("identb", [128, 128], bft, isOutput=False)
    out = nc.declare_dram_parameter("out", [O, NPIX], bft, isOutput=True)

    with tile.TileContext(nc) as tc:
        with tc.tile_pool(name="const", bufs=1) as cpool:
            # wrap split into 4 tensors so the first gathers only wait for
            # the first quarter of the index upload
            wrap_sb = [cpool.tile([128, (NT // 4) * (NIDX_T // 16)], i16,
                                  tag=f"wrap{q}", name=f"wrap{q}")
                       for q in range(4)]
            NCQ = (NT // 4) * (NIDX_T // 16)
            for q in range(4):
                nc.sync.dma_start(out=wrap_sb[q][:],
                                  in_=wrap[:, q * NCQ:(q + 1) * NCQ])
            wt_sb = cpool.tile([C, K, O], bft, tag="wt")
            nc.sync.dma_start(out=wt_sb[:], in_=wt[:].rearrange("k c o -> c k o"))
            ib_sb = cpool.tile([128, 128], bft, tag="identb")
            nc.sync.dma_start(out=ib_sb[:], in_=identb[:])
            ac_sb = [cpool.tile([128, NT, KB], bft, tag=f"ac{i}",
                                name=f"ac{i}") for i in range(4)]
            for i in range(4):
                nc.sync.dma_start(out=ac_sb[i][:], in_=ac[i])
            # one-time 4-expansion (DVE is idle early; saves 4MB of DMA in
            # the gather-critical window vs uploading pre-expanded weights;
            # 4-wide keeps the step-1 innermost dim for DVE 2x mode)
            ae_sb = [cpool.tile([128, NT, KB, 4], bft, tag=f"ae{i}",
                                name=f"ae{i}") for i in range(4)]
            for i in range(4):
                nc.vector.tensor_copy(
                    out=ae_sb[i][:],
                    in_=ac_sb[i][:, :, :, None].to_broadcast(
                        [128, NT, KB, 4]))

            # xt2 viewed as overlapping 2-row chunks: idx i -> rows (i, i+1)
            xtap = xt2[:]
            xt2_pair = bass.AP(xtap.tensor, 0, [[2 * C, XT2ROWS - 1], [1, 4 * C]])

            with (
                tc.tile_pool(name="gat", bufs=3) as gpool,
                tc.tile_pool(name="glast", bufs=1) as glpool,
                tc.tile_pool(name="pat", bufs=2) as ppool,
                tc.tile_pool(name="ost", bufs=3) as opool,
                tc.tile_pool(name="tpsum", bufs=3, space="PSUM") as tpsum,
                tc.tile_pool(name="opsum", bufs=2, space="PSUM") as opsum,
            ):
                _qn = [0]
                SUBC = (8, 8, 8, 8, 4)       # kb-columns per gather call

                def scale_sections(gj, t, kb0, ncol):
                    gap = gj[:]
                    for i in range(4):
                        axap = ae_sb[i][:]
                        in1 = bass.AP(axap.tensor,
                                      axap.offset + t * (KB * 4) + kb0 * 4,
                                      [[axap.ap[0][0], 128], [4, ncol],
                                       [0, C // 4], [1, 4]])
                        sec = bass.AP(gap.tensor, gap.offset + i * C,
                                      [[gap.ap[0][0], 128], [4 * C, ncol],
                                       [4, C // 4], [1, 4]])
                        nc.vector.tensor_tensor(out=sec, in0=sec, in1=in1,
                                                op=Alu.mult)

                for t in range(NT):
                    last = (t == NT - 1)
                    # ucode caps dma_gather at 1024 idxs/call -> 5 calls
                    # (4x1024 + 512) per tile. The last tile uses per-call
                    # sub-tiles so its tail latency is one call, not five.
                    gs = []
                    kb0 = 0
                    for j, ncol in enumerate(SUBC):
                        if last:
                            gj = glpool.tile([128, ncol, 4 * C], bft,
                                             tag=f"gl{j}", name=f"gl{j}")
                        elif j == 0:
                            gj = gpool.tile([128, KB, 4 * C], bft, tag="g")
                        nidx = ncol * 128
                        c0 = (t % (NT // 4)) * (NIDX_T // 16) + kb0 * 8
                        out_ap = gj[:] if last else gj[:, kb0:kb0 + ncol]
                        nc.gpsimd.dma_gather(
                            out_ap=out_ap,
                            in_ap=xt2_pair,
                            idxs_ap=wrap_sb[t // (NT // 4)][:, c0:c0 + nidx // 16],
                            num_idxs=nidx, num_idxs_reg=nidx,
                            elem_size=4 * C, elem_step=2 * C,
                            queue_num=_qn[0] % 4,
                        )
                        _qn[0] += 1
                        if last:
                            scale_sections(gj, t, kb0, ncol)
                            gs.append(gj)
                        kb0 += ncol
                    if not last:
                        scale_sections(gj, t, 0, KB)

                    patches = ppool.tile([C, K, TPIX], bft, tag="patches")
                    for k in range(K):
                        pp = tpsum.tile([128, TBLK, 128], f32, tag="pp")
                        for b in range(TBLK):
                            kb = k * TBLK + b
                            if last:
                                gsl = gs[4] if kb >= 32 else gs[kb // 8]
                                col = kb % 8 if kb < 32 else kb - 32
                                lhsT = gsl[:, col]
                            else:
                                lhsT = gj[:, kb]
                            for i in range(4):
                                nc.tensor.matmul(
                                    out=pp[:, b, :],
                                    lhsT=lhsT[:, i * C:(i + 1) * C],
                                    rhs=ib_sb[:],
                                    start=(i == 0), stop=(i == 3),
                                )
                        nc.scalar.copy(out=patches[:, k, :],
                                       in_=pp[:].rearrange("p b n -> p (b n)"))

                    op_ = opsum.tile([O, TPIX], f32, tag="op")
                    for k in range(K):
                        nc.tensor.matmul(
                            out=op_[:],
                            lhsT=wt_sb[:, k, :],
                            rhs=patches[:, k, :],
                            start=(k == 0), stop=(k == K - 1),
                        )
                    o_sb = opool.tile([O, TPIX], bft, tag="o_sb")
                    nc.scalar.copy(out=o_sb[:], in_=op_[:])
                    nc.sync.dma_start(out=out[:, t * TPIX:(t + 1) * TPIX],
                                      in_=o_sb[:])

    nc.finalize()
    return nc


def _host_prep_core(x_b, off_bh, h2):
    """Index + weight prep for one core. off_bh: [2K, HALF, W] f32."""
    kk = np.arange(K)
    ky = (kk // 3 - 1).astype(np.float32)          # [-1, 0, 1] row-major taps
    kx = (kk % 3 - 1).astype(np.float32)

    hh = (h2 * HALF + np.arange(HALF, dtype=np.float32))   # [64] output rows
    ww = np.arange(W, dtype=np.float32)                    # [128] output cols

    oy = off_bh[0::2]                               # [K, 64, 128]
    ox = off_bh[1::2]
    py = hh[None, :, None] + ky[:, None, None] + oy
    px = ww[None, None, :] + kx[:, None, None] + ox

    y0 = np.floor(py)
    x0 = np.floor(px)
    wy = py - y0
    wx = px - x0

    vy0 = (y0 >= 0) & (y0 <= H - 1)
    vy1 = (y0 >= -1) & (y0 <= H - 2)
    vx0 = (x0 >= 0) & (x0 <= W - 1)
    vx1 = (x0 >= -1) & (x0 <= W - 2)

    c0 = (1.0 - wy) * vy0
    c1 = wy * vy1
    b0 = (1.0 - wx) * vx0
    b1 = wx * vx1
    # section order matches the gathered chunk: TL, BL, TR, BR
    a_sec = np.stack([c0 * b0, c1 * b0, c0 * b1, c1 * b1])  # [4, K, 64, 128]

    idx = (np.clip(y0, -1.0, 127.0) * W + np.clip(x0, -1.0, 128.0)
           + XQPAD).astype(np.int32)
    idx = np.clip(idx, 0, XT2ROWS - 3)              # [K, 64, 128]

    # wrapped int16 idx layout: per tile, j = (k*TBLK+b)*128 + p,
    # stored at [j%16 (replicated over 8 bands), t*288 + j//16]
    idx_t = idx.reshape(K, NT, TBLK, W).transpose(1, 0, 2, 3)  # [NT,K,TBLK,128]
    flat = idx_t.reshape(NT, NIDX_T)
    w16 = flat.reshape(NT, NIDX_T // 16, 16).transpose(0, 2, 1)  # [NT,16,288]
    wrap16 = w16.transpose(1, 0, 2).reshape(16, NT * (NIDX_T // 16))
    wrap = np.tile(wrap16, (8, 1)).astype(np.int16)  # [128, NT*288]

    # ac[i][p, t, k*TBLK+b] = a_sec[i, k, t*TBLK+b, p]
    acm = a_sec.reshape(4, K, NT, TBLK, W).transpose(0, 4, 2, 1, 3)
    acm = np.ascontiguousarray(acm.reshape(4, 128, NT, KB)).astype(bf16)
    return wrap, acm


def _host_inputs(x, offset, weight):
    wT = np.ascontiguousarray(
        weight.reshape(O, C, K).transpose(2, 1, 0)).astype(bf16)  # [k, c, o]
    identb = np.eye(128, dtype=np.float32).astype(bf16)

    xt2s = []
    for b in range(B):
        xq = np.zeros((XT2ROWS + 130, C), bf16)
        xq[XQPAD:XQPAD + HW] = x[b].reshape(C, HW).T.astype(bf16)
        xt2 = np.concatenate([xq[:XT2ROWS], xq[128:128 + XT2ROWS]], axis=1)
        xt2s.append(np.ascontiguousarray(xt2))

    in_maps = []
    meta = []
    for b in range(B):
        for h2 in range(2):
            off_bh = np.ascontiguousarray(
                offset[b, :, h2 * HALF:(h2 + 1) * HALF, :])
            wrap, acm = _host_prep_core(x[b], off_bh, h2)
            in_maps.append({
                "xt2": xt2s[b], "wrap": wrap, "ac": acm,
                "wt": wT, "identb": identb,
            })
            meta.append((b, h2))
    return in_maps, meta


def _run(in_maps, trace=False):
    from concourse.bass_utils import run_bass_kernel_spmd

    if "nc" not in _CACHE:
        _CACHE["nc"] = _build_nc()
    nc = _CACHE["nc"]
    return run_bass_kernel_spmd(nc, in_maps, list(range(8)), trace=trace)


def kernel(x, offset, weight):
    x = np.asarray(x, dtype=np.float32)
    offset = np.asarray(offset, dtype=np.float32)
    weight = np.asarray(weight, dtype=np.float32)
    in_maps, meta = _host_inputs(x, offset, weight)
    res = _run(in_maps, trace=bool(int(os.environ.get("DEFORM_TRACE", "0"))))
    _CACHE["last_result"] = res
    out = np.zeros((B, O, H, W), np.float32)
    for i, (b, h2) in enumerate(meta):
        out[b, :, h2 * HALF:(h2 + 1) * HALF, :] = \
            np.asarray(res.results[i]["out"]).reshape(O, HALF, W)
    return out
